# revision 50
# baseline (speedup 1.0000x reference)
"""AutoCorrelationLayer Trainium2 kernel: 8 NeuronCores, data-parallel over batch.

Two launches plus a small host refinement step:
  L1 (per core, 2 batches): fold t <-> L-1-t pre-projection (y = x + x_rev,
     z = x - x_rev, on GpSimd) -> exact fp32 projections (dumped to DRAM for
     the host) -> half-length real DFT in fp8(e4m3) with DoubleRow matmuls
     via half-sample-shifted cos/sin bases (Q = e^{i pi f/L} (u1 - i u2);
     the rotation cancels in Q conj(K)) -> cross-spectrum (scaled 1/256 into
     fp8) -> inverse half-DFT + tau-mirror (fp8 DoubleRow, G prescaled 1024)
     -> streaming per-channel top-8 of each finished tau region (48+1
     noise-proof candidates per channel).
  host: reconstruct exact fp32 Q,K from the y/z dumps, compute exact
     autocorr values at the 49 candidate lags, re-rank -> exact top-8,
     global shifts (floor of mean) + softmax weights. fp8 noise only
     affects candidate selection, never the shift/weight values
     (validated: the candidate regions capture the true top-8).
  L2 (per core): same fold for V (f32r) -> half-length forward DFT ->
     multiply by M''[f,c] = sum_k w_k[c] e^{2 pi i f (s_k+1/2) / L} (host
     twiddles; the +1/2 absorbs the fold rotation) -> inverse DFT
     == sum_k w_k * roll(V, -s_k) -> output projection (f32r agg read
     directly by the out-projection matmuls, grouped output DMAs).

Projections in exact fp32 (4 cyc/row); candidate DFT chain in fp8 DoubleRow
(0.5 cyc/row); V-path in f32r (1 cyc/row). PSUM banks alternate across
frequency tiles; cross-spectrum temps staged to SBUF so banks free early.
SBUF tiles are [128, ntile, ...] (partition dim <= 128).
"""
import numpy as np

from concourse import bass, bacc, mybir, tile
from concourse.bass_utils import run_bass_kernel_spmd

import ml_dtypes

f32 = mybir.dt.float32
f32r = mybir.dt.float32r
bf16 = mybir.dt.bfloat16
f8 = mybir.dt.float8e4
u32 = mybir.dt.uint32
bfnp = ml_dtypes.bfloat16
e4np = ml_dtypes.float8_e4m3
DR = mybir.MatmulPerfMode.DoubleRow
GSC = 1024.0  # G-matrix prescale (fp8 range); PSC = cross-spectrum downscale
PSC = 1.0 / 256.0


def _round11(x):
    """truncate fp32 mantissa to 11 bits (f32r-representable values)."""
    x = np.ascontiguousarray(x, np.float32)
    iv = x.view(np.uint32)
    mask = np.uint32(0xFFFFFFFF) << np.uint32(12)
    return (iv & mask).view(np.float32).copy()

B, L, D, H = 16, 3072, 512, 8
NCORE = 8
BPC = B // NCORE
F = L // 2 + 1  # 1537
FP = 1664  # 13*128
LH = L // 2  # 1536 folded time length
NT = LH // 128  # 12 folded t-tiles
NF = FP // 128  # 13
NC = D // 128  # 4
TAU_CHUNKS = [(0, 512), (512, 512), (1024, 512), (1536, 1)]
ADD = mybir.AluOpType.add
SUB = mybir.AluOpType.subtract
MUL = mybir.AluOpType.mult


def _build_static():
    t = np.arange(LH, dtype=np.float64)[:, None] + 0.5
    f = np.arange(FP, dtype=np.float64)[None, :]
    ang = 2.0 * np.pi * t * f / L
    M1 = np.cos(ang)
    M2 = np.sin(ang)
    M1[:, F:] = 0.0
    M2[:, F:] = 0.0
    wgt = np.full(FP, 2.0)
    wgt[0] = 1.0
    wgt[1536] = 1.0
    wgt[F:] = 0.0
    tau = np.arange(F, dtype=np.float64)[None, :]
    fv = np.arange(FP, dtype=np.float64)[:, None]
    ang2 = 2.0 * np.pi * fv * tau / L
    Gc = (wgt[:, None] / L) * np.cos(ang2)
    Gs = -(wgt[:, None] / L) * np.sin(ang2)
    ident = np.eye(128, dtype=np.float32)
    return (
        np.ascontiguousarray(M1, np.float32),
        np.ascontiguousarray(M2, np.float32),
        np.ascontiguousarray(Gc, np.float32),
        np.ascontiguousarray(Gs, np.float32),
        ident,
    )


_STATIC = None


def _static():
    global _STATIC
    if _STATIC is None:
        _STATIC = _build_static()
    return _STATIC


def _row_major(ap2d):
    """view DRAM [R, C] (R = a*128 + p) as [p, a, C]."""
    return ap2d.rearrange("(a p) c -> p a c", p=128)


def _fold_project(
    nc, stream, ps, ident_t, fwd3, rev3, w_ap, Y, Z, dumpY, dumpZ, dt_mm=f32,
    xcopy=None,
):
    """Per folded t-tile: load x[t] and x[L-1-t] rows, form y = sum, z = diff
    (fp32, exact), PE-transpose, project with w (fp32 exact), write Y/Z
    (f32r rounding on the SBUF copy) and optionally dump exact pp to DRAM."""
    for tt in range(NT):
        xin = stream.tile([128, D], f32, tag="xin")
        nc.sync.dma_start(xin[:], fwd3[:, tt, :])
        xrv = stream.tile([128, D], f32, tag="xrv")
        nc.sync.dma_start(xrv[:], rev3[:, tt, :])
        for sgn, X, dump3 in ((ADD, Y, dumpY), (SUB, Z, dumpZ)):
            xf = stream.tile([128, D], f32, tag="xf")
            nc.gpsimd.tensor_tensor(xf[:], xin[:], xrv[:], sgn)
            xcol = stream.tile([128, NC, 128], dt_mm, tag="xcol")
            for jt in range(NC):
                pt = ps.tile([128, 128], f32, tag="mmA")
                nc.tensor.transpose(
                    pt[:], xf[:, 128 * jt : 128 * (jt + 1)], ident_t[:]
                )
                (xcopy or nc.vector.tensor_copy)(xcol[:, jt, :], pt[:])
            pp = ps.tile([128, D], f32, tag="mmB")
            for jt in range(NC):
                nc.tensor.matmul(
                    pp[:],
                    xcol[:, jt, :],
                    w_ap[:, jt, :],
                    start=(jt == 0),
                    stop=(jt == NC - 1),
                )
            if dump3 is not None:
                stg = stream.tile([128, D], f32, tag="stg")
                nc.scalar.copy(stg[:], pp[:])
                nc.sync.dma_start(dump3[:, tt, :], stg[:])
            nc.vector.tensor_copy(X[:, tt, :], pp[:])


CHUNK_REGIONS = {0: [(0, 0, 512), (1, 2561, 511)],
                 1: [(2, 512, 512), (3, 2049, 512)],
                 2: [(4, 1024, 512), (5, 1537, 512)]}


def _inverse(nc, ps, psF, stream, Pr, Pi, gc_d, gs_d, dsts, dt_mm=f32r,
             topk_cb=None, deferred=None):
    """dsts: list of (tile, local_ct) covering NC channel-tiles.
    dst[c, 0..1536] = u+v ; dst[c, L-tau] = u-v. Chunk-major with all NC
    channel-tiles accumulating at once (8 PSUM banks) so each G block is
    streamed exactly once per batch."""
    PSUM_TAGS = [
        (psF, "pQr"), (psF, "pQi"), (psF, "pKr"), (psF, "pKi"),
        (ps, "mmB"), (ps, "mmB"), (ps, "mmA"), (ps, "mmA"),
    ]
    for ci, (t0, tw) in enumerate(TAU_CHUNKS):
        pus = []
        pvs = []
        for ct in range(NC):
            pool_u, tag_u = PSUM_TAGS[2 * ct]
            pool_v, tag_v = PSUM_TAGS[2 * ct + 1]
            pu = pool_u.tile([128, 512], f32, tag=tag_u)
            pv = pool_v.tile([128, 512], f32, tag=tag_v)
            pus.append(pu)
            pvs.append(pv)
        if dt_mm == f8:
            ghalf = (NF - 1) // 2  # 6 f-tile pairs + 1 trailing single
            for gi in range(ghalf + 1):
                nrow = 2 if gi < ghalf else 1
                fr = slice(256 * gi, 256 * gi + 128 * nrow)
                gcb = stream.tile([128, 2, 512], dt_mm, tag="gcb")
                gsb = stream.tile([128, 2, 512], dt_mm, tag="gsb")
                gsrc = gc_d.ap()[fr, t0 : t0 + tw]
                nc.sync.dma_start(
                    gcb[:, :nrow, :tw],
                    gsrc.rearrange("(a p) c -> p a c", p=128),
                )
                gsrc2 = gs_d.ap()[fr, t0 : t0 + tw]
                nc.sync.dma_start(
                    gsb[:, :nrow, :tw],
                    gsrc2.rearrange("(a p) c -> p a c", p=128),
                )
                for ct in range(NC):
                    csl = slice(128 * ct, 128 * (ct + 1))
                    dr_ok = tw >= 256 and nrow == 2
                    for PP, gb, acc in ((Pr, gcb, pus), (Pi, gsb, pvs)):
                        if dr_ok:
                            nc.tensor.matmul(
                                acc[ct][:, :tw],
                                PP[:, 2 * gi : 2 * gi + 2, csl],
                                gb[:, :, :tw],
                                start=(gi == 0), stop=False,
                                perf_mode=DR,
                            )
                        else:
                            for j in range(nrow):
                                ft = 2 * gi + j
                                nc.tensor.matmul(
                                    acc[ct][:, :tw],
                                    PP[:, ft, csl],
                                    gb[:, j, :tw],
                                    start=(ft == 0), stop=(ft == NF - 1),
                                )
        else:
            for ft in range(NF):
                fsl = slice(128 * ft, 128 * (ft + 1))
                gcb = stream.tile([128, 1, 512], dt_mm, tag="gcb")
                gsb = stream.tile([128, 1, 512], dt_mm, tag="gsb")
                nc.sync.dma_start(gcb[:, 0, :tw], gc_d.ap()[fsl, t0 : t0 + tw])
                nc.sync.dma_start(gsb[:, 0, :tw], gs_d.ap()[fsl, t0 : t0 + tw])
                for ct in range(NC):
                    lr = Pr[:, ft, 128 * ct : 128 * (ct + 1)]
                    li = Pi[:, ft, 128 * ct : 128 * (ct + 1)]
                    rc = gcb[:, 0, :tw]
                    rs = gsb[:, 0, :tw]
                    if tw < 256 and dt_mm == f32r:
                        lr, li = lr.bitcast(f32), li.bitcast(f32)
                        rc, rs = rc.bitcast(f32), rs.bitcast(f32)
                    nc.tensor.matmul(
                        pus[ct][:, :tw], lr, rc, start=(ft == 0), stop=(ft == NF - 1)
                    )
                    nc.tensor.matmul(
                        pvs[ct][:, :tw], li, rs, start=(ft == 0), stop=(ft == NF - 1)
                    )
        for ct in range(NC):
            dst, lct = dsts[ct]
            pu, pv = pus[ct], pvs[ct]
            nc.scalar.copy(dst[:, lct, t0 : t0 + tw], pu[:, :tw])
            tt_eng = nc.vector
            tt_eng.tensor_tensor(
                dst[:, lct, t0 : t0 + tw],
                dst[:, lct, t0 : t0 + tw],
                pv[:, :tw],
                ADD,
            )
            if t0 == 0:
                tt_eng.scalar_tensor_tensor(
                    dst[:, lct, L - 511 : L][:, ::-1],
                    pv[:, 1:512],
                    -2.0,
                    dst[:, lct, 1:512],
                    MUL,
                    ADD,
                )
            elif tw == 512:
                tt_eng.scalar_tensor_tensor(
                    dst[:, lct, L - t0 - 511 : L - t0 + 1][:, ::-1],
                    pv[:, :tw],
                    -2.0,
                    dst[:, lct, t0 : t0 + tw],
                    MUL,
                    ADD,
                )
        if topk_cb is not None and ci in CHUNK_REGIONS:
            if ci == 2 and deferred is not None:
                for ct in range(NC):
                    dst, lct = dsts[ct]
                    deferred.append((ci, ct, dst, lct))
            else:
                for ct in range(NC):
                    dst, lct = dsts[ct]
                    topk_cb(ci, ct, dst, lct)


def _fwd_dft(nc, stream, psF, m1_d, m2_d, ft, pairs, dt_mm=f32r):
    """pairs: list of (Ytile, Ztile, psum_u1, psum_u2). u1 = M1^T y,
    u2 = M2^T z accumulated over the NT folded t-tiles. fp8 runs
    DoubleRow over t-tile pairs (2 contraction tiles per instruction)."""
    fsl = slice(128 * ft, 128 * (ft + 1))
    m1b = stream.tile([128, NT, 128], dt_mm, tag="m1b")
    nc.sync.dma_start(m1b[:], _row_major(m1_d.ap())[:, :, fsl])
    m2b = stream.tile([128, NT, 128], dt_mm, tag="m2b")
    nc.sync.dma_start(m2b[:], _row_major(m2_d.ap())[:, :, fsl])
    for Y, Z, pu1, pu2 in pairs:
        for mb, X, pu in ((m1b, Y, pu1), (m2b, Z, pu2)):
            if dt_mm == f8:
                for i in range(NT // 2):
                    nc.tensor.matmul(
                        pu[:], mb[:, 2 * i : 2 * i + 2, :],
                        X[:, 2 * i : 2 * i + 2, :],
                        start=(i == 0), stop=(i == NT // 2 - 1),
                        perf_mode=DR,
                    )
            else:
                for tl in range(NT):
                    nc.tensor.matmul(
                        pu[:], mb[:, tl, :], X[:, tl, :],
                        start=(tl == 0), stop=(tl == NT - 1),
                    )


def _build_l1():
    nc = bacc.Bacc("TRN2", target_bir_lowering=False, debug=False)
    q_d = nc.dram_tensor("q", [BPC, L, D], f32, kind="ExternalInput")
    k_d = nc.dram_tensor("k", [BPC, L, D], f32, kind="ExternalInput")
    qr_d = nc.dram_tensor("qrev", [BPC, LH, D], f32, kind="ExternalInput")
    kr_d = nc.dram_tensor("krev", [BPC, LH, D], f32, kind="ExternalInput")
    wq_d = nc.dram_tensor("wq", [D, D], f32, kind="ExternalInput")
    wk_d = nc.dram_tensor("wk", [D, D], f32, kind="ExternalInput")
    m1_d = nc.dram_tensor("m1", [LH, FP], f8, kind="ExternalInput")
    m2_d = nc.dram_tensor("m2", [LH, FP], f8, kind="ExternalInput")
    gc_d = nc.dram_tensor("gc", [FP, F], f8, kind="ExternalInput")
    gs_d = nc.dram_tensor("gs", [FP, F], f8, kind="ExternalInput")
    ident_d = nc.dram_tensor("ident", [128, 128], f32, kind="ExternalInput")
    ti_d = nc.dram_tensor("top_idx", [BPC, D, 48], u32, kind="ExternalOutput")
    # y/z dumps: [BPC, 2 (y,z), LH, D] exact fp32 projections of the folds
    yq_d = nc.dram_tensor("yq", [BPC, 2, LH, D], f32, kind="ExternalOutput")
    yk_d = nc.dram_tensor("yk", [BPC, 2, LH, D], f32, kind="ExternalOutput")

    with tile.TileContext(nc) as tc:
        with (
            tc.tile_pool(name="stat", bufs=1) as stat,
            tc.tile_pool(name="work", bufs=1) as work,
            tc.tile_pool(name="stream", bufs=2) as stream,
            tc.tile_pool(name="streamF", bufs=3) as streamF,
            tc.tile_pool(name="psA", bufs=2, space="PSUM") as psA,
            tc.tile_pool(name="psF", bufs=1, space="PSUM") as psF,
        ):
            ident_t = stat.tile([128, 128], f32)
            nc.sync.dma_start(ident_t[:], ident_d.ap())
            Pr = work.tile([128, NF, D], f8, tag="Pr")
            Pi = work.tile([128, NF, D], f8, tag="Pi")
            wq_t = stat.tile([128, NC, D], f32)
            wk_t = stat.tile([128, NC, D], f32)
            for jt in range(NC):
                nc.sync.dma_start(wq_t[:, jt, :], _row_major(wq_d.ap())[:, jt, :])
            for jt in range(NC):
                nc.sync.dma_start(wk_t[:, jt, :], _row_major(wk_d.ap())[:, jt, :])

            pend = []
            pend_cb = [None]
            for b in range(BPC):
                Yq = work.tile([128, NT, D], f8, tag="Yq")
                Zq = work.tile([128, NT, D], f8, tag="Zq")
                Yk = work.tile([128, NT, D], f8, tag="Yk")
                Zk = work.tile([128, NT, D], f8, tag="Zk")
                for x_d, xr_d, w_ap, Y, Z, dump_d in (
                    (q_d, qr_d, wq_t[:], Yq, Zq, yq_d),
                    (k_d, kr_d, wk_t[:], Yk, Zk, yk_d),
                ):
                    _fold_project(
                        nc, stream, psA, ident_t,
                        _row_major(x_d.ap()[b]),
                        _row_major(xr_d.ap()[b]),
                        w_ap, Y, Z,
                        _row_major(dump_d.ap()[b][0]),
                        _row_major(dump_d.ap()[b][1]),
                    )  # xcol copies stay on DVE here (Act is busy with dumps)

                if pend:
                    for args in pend:
                        pend_cb[0](*args)
                    pend = []

                for ft in range(NF):
                    # alternate PSUM banks across ft so the next ft's
                    # accumulation never waits on this ft's DVE reads
                    if ft % 2 == 0:
                        pQ1 = psF.tile([128, D], f32, tag="pQr")
                        pQ2 = psF.tile([128, D], f32, tag="pQi")
                        pK1 = psF.tile([128, D], f32, tag="pKr")
                        pK2 = psF.tile([128, D], f32, tag="pKi")
                    else:
                        pQ1 = psA.tile([128, D], f32, tag="mmA")
                        pQ2 = psA.tile([128, D], f32, tag="mmA")
                        pK1 = psA.tile([128, D], f32, tag="mmB")
                        pK2 = psA.tile([128, D], f32, tag="mmB")
                    _fwd_dft(
                        nc, streamF, psF, m1_d, m2_d, ft,
                        [(Yq, Zq, pQ1, pQ2), (Yk, Zk, pK1, pK2)],
                        dt_mm=f8,
                    )
                    # P = (u1 - i u2) (w1 + i w2), prescaled by PSC on the
                    # PSUM->SBUF copies: Pr = u1 w1 + u2 w2 ; Pi = u1 w2 - u2 w1
                    qr = work.tile([128, D], f32, tag="qr")
                    qi = work.tile([128, D], f32, tag="qi")
                    kr = work.tile([128, D], f32, tag="kr")
                    ki = work.tile([128, D], f32, tag="ki")
                    nc.scalar.mul(qr[:], pQ1[:], PSC)
                    nc.scalar.mul(qi[:], pQ2[:], PSC)
                    nc.scalar.copy(kr[:], pK1[:])
                    nc.scalar.copy(ki[:], pK2[:])
                    t1 = work.tile([128, D], f32, tag="t1")
                    t2 = work.tile([128, D], f32, tag="t2")
                    nc.gpsimd.tensor_tensor(t1[:], qi[:], ki[:], MUL)
                    nc.vector.tensor_tensor(Pr[:, ft, :], qr[:], kr[:], MUL)
                    nc.vector.tensor_tensor(Pr[:, ft, :], Pr[:, ft, :], t1[:], ADD)
                    nc.gpsimd.tensor_tensor(t2[:], qi[:], kr[:], MUL)
                    nc.vector.tensor_tensor(Pi[:, ft, :], qr[:], ki[:], MUL)
                    nc.vector.tensor_tensor(Pi[:, ft, :], Pi[:, ft, :], t2[:], SUB)

                ac1 = work.tile([128, 2, L], bf16, tag="ac1")
                ac2 = work.tile([128, 2, L], bf16, tag="ac2")
                dsts = [(ac1, 0), (ac1, 1), (ac2, 0), (ac2, 1)]
                tits = []
                tvts = []
                for ct in range(NC):
                    tit = work.tile([128, 48], u32, tag=f"tit{ct}")
                    tvt = work.tile([128, 8], bf16, tag=f"tvt{ct}")
                    tits.append(tit)
                    tvts.append(tvt)

                def topk_cb(ci, ct, dst, lct, b=b, tits=tits, tvts=tvts):
                    for r, start, width in CHUNK_REGIONS[ci]:
                        reg = dst[:, lct, start : start + width]
                        nc.vector.max(tvts[ct][:], reg)
                        nc.vector.max_index(
                            tits[ct][:, 8 * r : 8 * (r + 1)], tvts[ct][:], reg
                        )
                    if ci == 2:
                        nc.sync.dma_start(
                            _row_major(ti_d.ap()[b])[:, ct, :], tits[ct][:]
                        )

                pend_cb[0] = topk_cb
                _inverse(nc, psA, psF, streamF, Pr, Pi, gc_d, gs_d, dsts,
                         dt_mm=f8, topk_cb=topk_cb, deferred=pend)

            for args in pend:
                pend_cb[0](*args)

    nc.compile()
    return nc


def _build_l2():
    nc = bacc.Bacc("TRN2", target_bir_lowering=False, debug=False)
    v_d = nc.dram_tensor("v", [BPC, L, D], f32, kind="ExternalInput")
    vr_d = nc.dram_tensor("vrev", [BPC, LH, D], f32, kind="ExternalInput")
    wv_d = nc.dram_tensor("wv", [D, D], f32r, kind="ExternalInput")
    wo_d = nc.dram_tensor("wo", [D, D], f32r, kind="ExternalInput")
    m1_d = nc.dram_tensor("m1", [LH, FP], f32r, kind="ExternalInput")
    m2_d = nc.dram_tensor("m2", [LH, FP], f32r, kind="ExternalInput")
    gc_d = nc.dram_tensor("gc", [FP, F], f32r, kind="ExternalInput")
    gs_d = nc.dram_tensor("gs", [FP, F], f32r, kind="ExternalInput")
    ident_d = nc.dram_tensor("ident", [128, 128], f32, kind="ExternalInput")
    wts_d = nc.dram_tensor("wts", [BPC, 8, D], f32r, kind="ExternalInput")
    ec_d = nc.dram_tensor("ec", [8, FP], f32r, kind="ExternalInput")
    es_d = nc.dram_tensor("es", [8, FP], f32r, kind="ExternalInput")
    out_d = nc.dram_tensor("out", [BPC, L, D], f32, kind="ExternalOutput")

    with tile.TileContext(nc) as tc:
        with (
            tc.tile_pool(name="stat", bufs=1) as stat,
            tc.tile_pool(name="work", bufs=1) as work,
            tc.tile_pool(name="stream", bufs=2) as stream,
            tc.tile_pool(name="streamF", bufs=3) as streamF,
            tc.tile_pool(name="psA", bufs=2, space="PSUM") as psA,
            tc.tile_pool(name="psF", bufs=1, space="PSUM") as psF,
        ):
            ident_t = stat.tile([128, 128], f32)
            nc.sync.dma_start(ident_t[:], ident_d.ap())
            Vtr = work.tile([128, NF, D], f32r, tag="Vtr")
            Vti = work.tile([128, NF, D], f32r, tag="Vti")
            wv_t = Vtr[:, 0:NC, :]
            wo_t = stat.tile([128, NC, D], f32r)
            ec_t = stat.tile([8, FP], f32r)
            nc.sync.dma_start(ec_t[:], ec_d.ap())
            es_t = stat.tile([8, FP], f32r)
            nc.sync.dma_start(es_t[:], es_d.ap())

            for b in range(BPC):
                for jt in range(NC):
                    nc.sync.dma_start(
                        wv_t[:, jt, :], _row_major(wv_d.ap())[:, jt, :]
                    )
                Yv = work.tile([128, NT, D], f32r, tag="Yv")
                Zv = work.tile([128, NT, D], f32r, tag="Zv")
                _fold_project(
                    nc, stream, psA, ident_t,
                    _row_major(v_d.ap()[b]),
                    _row_major(vr_d.ap()[b]),
                    wv_t, Yv, Zv, None, None, dt_mm=f32r,
                    xcopy=nc.scalar.copy,
                )

                wts_t = work.tile([8, D], f32r, tag="wts")
                nc.sync.dma_start(wts_t[:], wts_d.ap()[b])
                if b == 0:
                    nc.sync.dma_start(wo_t[:], _row_major(wo_d.ap()))

                for ft in range(NF):
                    fsl = slice(128 * ft, 128 * (ft + 1))
                    pV1 = psF.tile(
                        [128, D], f32, tag=("pQr" if ft % 2 == 0 else "pKr")
                    )
                    pV2 = psF.tile(
                        [128, D], f32, tag=("pQi" if ft % 2 == 0 else "pKi")
                    )
                    _fwd_dft(
                        nc, streamF, psF, m1_d, m2_d, ft, [(Yv, Zv, pV1, pV2)]
                    )
                    pMr = psA.tile([128, D], f32, tag="mmA")
                    pMi = psA.tile([128, D], f32, tag="mmA")
                    nc.tensor.matmul(
                        pMr[:], ec_t[:, fsl], wts_t[:], start=True, stop=True,
                    )
                    nc.tensor.matmul(
                        pMi[:], es_t[:, fsl], wts_t[:], start=True, stop=True,
                    )
                    # P = (u1 - i u2)(Mr + i Mi):
                    #   Pr = u1 Mr + u2 Mi ; Pi = u1 Mi - u2 Mr
                    vr = work.tile([128, D], f32, tag="qr")
                    vi = work.tile([128, D], f32, tag="qi")
                    nc.scalar.copy(vr[:], pV1[:])
                    nc.scalar.copy(vi[:], pV2[:])
                    t1 = work.tile([128, D], f32, tag="t1")
                    nc.vector.tensor_tensor(Vtr[:, ft, :], vr[:], pMr[:], MUL)
                    nc.vector.tensor_tensor(t1[:], vi[:], pMi[:], MUL)
                    nc.vector.tensor_tensor(Vtr[:, ft, :], Vtr[:, ft, :], t1[:], ADD)
                    nc.vector.tensor_tensor(Vti[:, ft, :], vr[:], pMi[:], MUL)
                    nc.vector.tensor_tensor(t1[:], vi[:], pMr[:], MUL)
                    nc.vector.tensor_tensor(Vti[:, ft, :], Vti[:, ft, :], t1[:], SUB)

                ag1 = work.tile([128, 2, L], f32r, tag="Yv")
                ag2 = work.tile([128, 2, L], f32r, tag="Zv")
                dsts = [(ag1, 0), (ag1, 1), (ag2, 0), (ag2, 1)]
                _inverse(nc, psA, psF, streamF, Vtr, Vti, gc_d, gs_d, dsts)

                for tg in range(L // 128 // 3):
                    ot3 = stream.tile([128, 3, D], f32, tag="ot3")
                    for tl in range(3):
                        tt = 3 * tg + tl
                        po = psA.tile([128, D], f32, tag="mmB")
                        for ct in range(NC):
                            dst, lct = dsts[ct]
                            nc.tensor.matmul(
                                po[:],
                                dst[:, lct, 128 * tt : 128 * (tt + 1)],
                                wo_t[:, ct, :],
                                start=(ct == 0),
                                stop=(ct == NC - 1),
                            )
                        nc.scalar.copy(ot3[:, tl, :], po[:])
                    nc.sync.dma_start(
                        _row_major(out_d.ap()[b])[:, 3 * tg : 3 * (tg + 1), :],
                        ot3[:],
                    )

    nc.compile()
    return nc


_L1 = None
_L2 = None
_last_shifts = None


def kernel(query, key, value, Wq, bq, Wk, bk, Wv, bv, Wo, bo):
    global _L1, _L2
    for bias in (bq, bk, bv, bo):
        assert np.max(np.abs(np.asarray(bias))) == 0.0, "nonzero biases unsupported"
    query = np.ascontiguousarray(np.asarray(query, np.float32))
    key = np.ascontiguousarray(np.asarray(key, np.float32))
    value = np.ascontiguousarray(np.asarray(value, np.float32))
    M1, M2, Gc, Gs, ident = _static()

    if _L1 is None:
        _L1 = _build_l1()
    if _L2 is None:
        _L2 = _build_l2()

    common1 = dict(
        wq=np.ascontiguousarray(np.asarray(Wq, np.float32).T),
        wk=np.ascontiguousarray(np.asarray(Wk, np.float32).T),
        m1=M1.astype(e4np), m2=M2.astype(e4np),
        gc=(GSC * Gc).astype(e4np), gs=(GSC * Gs).astype(e4np),
        ident=ident,
    )
    qrev = np.ascontiguousarray(query[:, :LH - 1 - L:-1])
    krev = np.ascontiguousarray(key[:, :LH - 1 - L:-1])
    in_maps1 = [
        {
            "q": query[BPC * c : BPC * (c + 1)],
            "k": key[BPC * c : BPC * (c + 1)],
            "qrev": qrev[BPC * c : BPC * (c + 1)],
            "krev": krev[BPC * c : BPC * (c + 1)],
            **common1,
        }
        for c in range(NCORE)
    ]
    r1 = run_bass_kernel_spmd(_L1, in_maps1, list(range(NCORE)))
    cand = np.concatenate([r["top_idx"] for r in r1.results], 0).astype(np.int64)
    REGION_STARTS = [0, 2561, 512, 2049, 1024, 1537]
    for r, st in enumerate(REGION_STARTS):  # top-8 of each finished tau region
        cand[..., 8 * r : 8 * (r + 1)] += st
    cand = np.concatenate(
        [cand, np.full((B, D, 1), 1536, np.int64)], axis=-1
    )  # + the tau=1536 singleton
    Yq = np.concatenate([r["yq"] for r in r1.results], 0)  # [B, 2, LH, D]
    Yk = np.concatenate([r["yk"] for r in r1.results], 0)

    # reconstruct exact fp32 projections: Q[t] = (y+z)/2, Q[L-1-t] = (y-z)/2
    def _recon(Yx):
        X = np.empty((B, L, D), np.float32)
        X[:, :LH] = 0.5 * (Yx[:, 0] + Yx[:, 1])
        X[:, LH:] = (0.5 * (Yx[:, 0] - Yx[:, 1]))[:, ::-1]
        return X

    Qp = _recon(Yq)
    Kp = _recon(Yk)

    # exact candidate autocorr values: vals[b,c,j] = sum_t Q[(t+tau)%L,c] K[t,c]
    vals = np.empty((B, D, 49), np.float32)
    tgrid = np.arange(L)[:, None]
    cgrid = np.arange(D)[None, :]
    for b in range(B):
        Qb, Kb = Qp[b], Kp[b]
        for j in range(49):
            idx = (tgrid + cand[b, :, j][None, :]) % L
            vals[b, :, j] = np.einsum(
                "tc,tc->c", Qb[idx, cgrid], Kb, optimize=True
            )

    order = np.argsort(-vals, axis=-1, kind="stable")[..., :8]  # [B, D, 8]
    top_idx = np.take_along_axis(cand, order, axis=-1)
    top_vals = np.take_along_axis(vals, order, axis=-1)

    shifts = np.floor(
        top_idx.reshape(B * D, 8).astype(np.float32).mean(axis=0, dtype=np.float32)
    ).astype(np.int64)
    global _last_shifts
    _last_shifts = shifts
    e = np.exp((top_vals - top_vals[..., :1]).astype(np.float32))
    wts = (e / e.sum(-1, keepdims=True)).astype(np.float32)
    wts_t = np.ascontiguousarray(np.transpose(wts, (0, 2, 1)))  # [B, 8, D]

    # host twiddles with the +1/2 shift that absorbs the fold rotation:
    # M''[f] = sum_k w_k e^{+2 pi i f (s_k + 1/2) / L}
    fgrid = np.arange(FP, dtype=np.float64)
    ang = 2.0 * np.pi * np.outer(shifts.astype(np.float64) + 0.5, fgrid) / L
    ec = np.cos(ang).astype(np.float32)
    es = np.sin(ang).astype(np.float32)
    ec[:, F:] = 0.0
    es[:, F:] = 0.0

    common2 = dict(
        wv=_round11(np.asarray(Wv, np.float32).T),
        wo=_round11(np.asarray(Wo, np.float32).T),
        m1=_round11(M1), m2=_round11(M2), gc=_round11(Gc), gs=_round11(Gs),
        ident=ident, ec=_round11(ec), es=_round11(es),
    )
    vrev = np.ascontiguousarray(value[:, :LH - 1 - L:-1])
    in_maps2 = [
        {
            "v": value[BPC * c : BPC * (c + 1)],
            "vrev": vrev[BPC * c : BPC * (c + 1)],
            "wts": _round11(wts_t[BPC * c : BPC * (c + 1)]),
            **common2,
        }
        for c in range(NCORE)
    ]
    r2 = run_bass_kernel_spmd(_L2, in_maps2, list(range(NCORE)))
    out = np.concatenate([r["out"] for r in r2.results], 0)
    return out.astype(np.float32)
